# revision 13
# baseline (speedup 1.0000x reference)
"""Trainium2 Bass kernel for symmetric ContextualLoss (nn_ContextualLoss).

Inputs (full, unsharded):
    source, target: [2, 128, 64, 64] float32
Output: scalar float32 (shape ()).

Math (per direction, per batch):
    s = source reshaped [ns=4096, c=128]; t likewise.
    dist[i, j] = ||t_i - s_j||^2
    m[j]  = min_i dist[i, j]
    E[i,j] = exp((1 - dist/(m[j]+eps)) / 0.5)
    Z[j]  = sum_i E[i, j]
    r[i]  = max_j E[i,j] / Z[j]
    sim   = mean_i r[i];  loss_dir = mean_b(-log sim)
    out = (loss(s,t) + loss(t,s)) / 2

Sharding: for each of the 4 (batch, direction) calls the distance matrix is
striped by columns across 8 cores (512 columns each).  Each core holds full
columns, so per-column min and Z are exact locally; only the final max over
columns is partial: the device reduces over its own 128-column q-tiles
(GpSimd partition_all_reduce) and the host finishes the max over
(cores x qtiles).

Device layout per (call, qtile): D[q=128 cols (partitions), i=4096 rows
(free)].  PSUM gets asq[i] (K=1 seed matmuls, operands parked at partitions
0/64) plus the K=128 main matmul with X pre-scaled by -2.  PSUM is drained
in halves: ScalarE Identity(+bsq bias) and VectorE tensor_scalar(+bsq,
min-accum) -> D fp16 in SBUF.  ScalarE Exp(scale=-2/(m+eps), bias=2)
accumulates Z; VectorE scales esc by 1/Z (4x mode); GpSimd
partition_all_reduce(max) gives the per-row max over the 128 local columns;
one staging row per qtile (partitions 0/32/64/96) is DMA'd out per call as
[4, 4096] fp16.
"""

import ml_dtypes
import numpy as np

import concourse.bacc as bacc
import concourse.bass as bass
import concourse.tile as tile
from concourse import bass_isa, mybir
from concourse.bass_utils import run_bass_kernel_spmd

N_CORES = 8
C = 128            # channels = matmul contraction dim
NP = 4096          # rows (i) per distance matrix
QS = 512           # column stripe per core
NQT = QS // 128    # 4 q part-tiles per stripe
NCALL = 4          # (batch, direction) pairs
EPS = 1e-5

F32 = mybir.dt.float32
F32R = mybir.dt.float32r
FP16 = mybir.dt.float16
BF16 = mybir.dt.bfloat16
ALU = mybir.AluOpType
ACT = mybir.ActivationFunctionType

LAST_RESULT = None  # BassKernelResults of the most recent run (for test harness)
_NC_CACHE = None


def _build_bass():
    nc = bacc.Bacc(
        "TRN2", target_bir_lowering=False, debug=False, num_devices=N_CORES
    )
    a_d = nc.dram_tensor("a", [NCALL, C, NP], BF16, kind="ExternalInput").ap()
    b2_d = nc.dram_tensor("b2", [NCALL, C, QS], BF16, kind="ExternalInput").ap()
    asq_d = nc.dram_tensor("asq", [NCALL, 2, NP // 2], F32R, kind="ExternalInput").ap()
    bsq_d = nc.dram_tensor("bsq", [NCALL, 128, NQT], F32, kind="ExternalInput").ap()
    ones_d = nc.dram_tensor("ones", [2, 128], F32R, kind="ExternalInput").ap()
    o_d = nc.dram_tensor("o", [NCALL, NQT, NP], FP16, kind="ExternalOutput").ap()

    with tile.TileContext(nc) as tc:
        with (
            tc.tile_pool(name="io", bufs=2) as io_pool,
            tc.tile_pool(name="const", bufs=1) as const_pool,
            tc.tile_pool(name="dtile", bufs=2) as d_pool,
            tc.tile_pool(name="etile", bufs=2) as e_pool,
            tc.tile_pool(name="stats", bufs=10) as st_pool,
            tc.tile_pool(name="stage", bufs=2) as stg_pool,
            tc.tile_pool(name="psum", bufs=2, space="PSUM") as ps_pool,
        ):
            ones = const_pool.tile([128, 128], F32R)
            nc.sync.dma_start(ones[0:128:64, :], ones_d[:])
            bias2 = const_pool.tile([128, 1], F32)
            nc.vector.memset(bias2[:], 2.0)

            for c in range(NCALL):
                # asq halves -> partitions 0 and 64 (strided-partition DMA);
                # small tensors first so seeds/drains can start early
                asq_sb = io_pool.tile([128, NP // 2], F32R, tag="asq")
                nc.sync.dma_start(asq_sb[0:128:64, :], asq_d[c])
                bsq_sb = io_pool.tile([128, NQT], F32, tag="bsq")
                nc.sync.dma_start(bsq_sb[:], bsq_d[c])
                b2_sb = io_pool.tile([C, QS], BF16, tag="b2")
                nc.sync.dma_start(b2_sb[:], b2_d[c])
                a_sb = io_pool.tile([C, NP], BF16, tag="a")
                nc.sync.dma_start(a_sb[:, 0 : NP // 2], a_d[c][:, 0 : NP // 2])
                nc.sync.dma_start(a_sb[:, NP // 2 : NP], a_d[c][:, NP // 2 : NP])

                stg = stg_pool.tile([128, NP], FP16, tag="stg")

                # ---- phase 1: matmuls, PSUM drains, per-column min ----
                dts, rm2s = [], []
                for qt in range(NQT):
                    dt_ = d_pool.tile([128, NP], FP16, tag=f"d{qt}")
                    dts.append(dt_)
                    mh = st_pool.tile([128, 2], F32, tag="mh")
                    for h in range(2):
                        ps = ps_pool.tile([128, NP // 2], F32, tag="ps")
                        bp = 0 if h == 0 else 64
                        for ch in range(4):
                            nc.tensor.matmul(
                                ps[:, ch * 512 : (ch + 1) * 512],
                                lhsT=ones[bp : bp + 1, :],
                                rhs=asq_sb[bp : bp + 1, ch * 512 : (ch + 1) * 512],
                                start=True,
                                stop=False,
                            )
                        for ch in range(4):
                            col0 = h * (NP // 2) + ch * 512
                            nc.tensor.matmul(
                                ps[:, ch * 512 : (ch + 1) * 512],
                                lhsT=b2_sb[:, qt * 128 : (qt + 1) * 128],
                                rhs=a_sb[:, col0 : col0 + 512],
                                start=False,
                                stop=True,
                            )
                        if h == 0:
                            # ScalarE drain: D = psum + bsq (fp16 out)
                            nc.scalar.activation(
                                dt_[:, 0 : NP // 2],
                                ps[:],
                                ACT.Identity,
                                bias=bsq_sb[:, qt : qt + 1],
                                scale=1.0,
                            )
                        else:
                            # VectorE drain: D = psum + bsq with min-accum
                            nc.vector.tensor_scalar(
                                dt_[:, NP // 2 : NP],
                                ps[:],
                                scalar1=bsq_sb[:, qt : qt + 1],
                                scalar2=None,
                                op0=ALU.add,
                                op1=ALU.min,
                                accum_out=mh[:, 1:2],
                            )
                    # min over the ScalarE half (4x fp16 pass, dummy out)
                    scr = e_pool.tile([128, NP // 2], FP16, tag="scr")
                    nc.vector.tensor_scalar(
                        scr[:],
                        dt_[:, 0 : NP // 2],
                        scalar1=1.0,
                        scalar2=None,
                        op0=ALU.mult,
                        op1=ALU.min,
                        accum_out=mh[:, 0:1],
                    )
                    # accum max of -mh/2 = -(min mh)/2;  rm2 = -2/m
                    # (eps=1e-5 is negligible against m ~ O(100))
                    vd = st_pool.tile([128, 2], F32, tag="vd")
                    v = st_pool.tile([128, 1], F32, tag="v")
                    nc.vector.tensor_scalar(
                        vd[:], mh[:], scalar1=-0.5, scalar2=None,
                        op0=ALU.mult, op1=ALU.max, accum_out=v[:],
                    )
                    rm2 = st_pool.tile([128, 1], F32, tag="rm2")
                    nc.vector.reciprocal(rm2[:], v[:])
                    rm2s.append(rm2)

                # ---- phase 2: exp/Z, normalize, per-row max, stage ----
                for qt in range(NQT):
                    esc = e_pool.tile([128, NP], FP16, tag="esc")
                    zc = st_pool.tile([128, 1], F32, tag="zc")
                    nc.scalar.activation(
                        esc[:], dts[qt][:], ACT.Exp,
                        bias=bias2[:], scale=rm2s[qt][:],
                        accum_out=zc[:],
                    )
                    zrec = st_pool.tile([128, 1], F32, tag="zrec")
                    nc.vector.reciprocal(zrec[:], zc[:])
                    # sim = esc / Z  (4x mode)
                    sim = e_pool.tile([128, NP], FP16, tag="sim")
                    nc.vector.tensor_scalar(
                        sim[:], esc[:], scalar1=zrec[:], scalar2=None,
                        op0=ALU.mult,
                    )
                    # per-row max over this qtile's 128 columns
                    par = e_pool.tile([128, NP], FP16, tag="par")
                    nc.gpsimd.partition_all_reduce(
                        par[:], sim[:], channels=128,
                        reduce_op=bass_isa.ReduceOp.max,
                    )
                    p0 = 32 * qt
                    nc.vector.tensor_copy(
                        stg[p0 : p0 + 1, :], par[p0 : p0 + 1, :]
                    )
                nc.sync.dma_start(o_d[c], stg[0:128:32, :])
    nc.compile()
    return nc


def kernel(source, target):
    global LAST_RESULT
    source = np.ascontiguousarray(np.asarray(source), dtype=np.float32)
    target = np.ascontiguousarray(np.asarray(target), dtype=np.float32)
    B = source.shape[0]
    s = source.reshape(B, C, NP)
    t = target.reshape(B, C, NP)

    # call order: (b0, dir s->t), (b0, dir t->s), (b1, ...), ...
    # dir s->t == _similarity(source, target): cols X = source, rows Y = target
    pairs = []
    for b in range(B):
        pairs.append((s[b], t[b]))
        pairs.append((t[b], s[b]))
    A = np.stack([y for (_x, y) in pairs])                      # [4, 128, 4096]
    X = np.stack([x for (x, _y) in pairs])                      # [4, 128, 4096]
    asq = (A.astype(np.float64) ** 2).sum(axis=1)               # [4, 4096]
    asq2 = np.ascontiguousarray(asq.reshape(NCALL, 2, NP // 2)).astype(np.float32)
    xsq = (X.astype(np.float64) ** 2).sum(axis=1)               # [4, 4096]
    A16 = A.astype(ml_dtypes.bfloat16)

    in_maps = []
    for k in range(N_CORES):
        sl = slice(k * QS, (k + 1) * QS)
        b2 = np.ascontiguousarray(-2.0 * X[:, :, sl]).astype(ml_dtypes.bfloat16)
        bsq = np.ascontiguousarray(
            xsq[:, sl].reshape(NCALL, NQT, 128).transpose(0, 2, 1)
        ).astype(np.float32)
        in_maps.append({
            "a": A16, "b2": b2, "asq": asq2, "bsq": bsq,
            "ones": np.ones((2, 128), dtype=np.float32),
        })

    global _NC_CACHE
    if _NC_CACHE is None:
        _NC_CACHE = _build_bass()
    nc = _NC_CACHE
    res = run_bass_kernel_spmd(nc, in_maps, core_ids=list(range(N_CORES)))
    LAST_RESULT = res

    # o: [cores][NCALL, NQT, NP] fp16 partial row-maxima
    outs = np.stack([r["o"].astype(np.float64) for r in res.results])
    r = outs.max(axis=(0, 2))                        # [4, 4096]
    sims = r.mean(axis=1)                            # [4]
    loss = float((-np.log(sims)).mean())
    return np.array(loss, dtype=np.float32)


# revision 45
# speedup vs baseline: 1.0746x; 1.0746x over previous
"""Trainium2 Bass kernel for symmetric ContextualLoss (nn_ContextualLoss).

Inputs (full, unsharded):
    source, target: [2, 128, 64, 64] float32
Output: scalar float32 (shape ()).

Math (per direction, per batch):
    s = source reshaped [ns=4096, c=128]; t likewise.
    dist[i, j] = ||t_i - s_j||^2
    m[j]  = min_i dist[i, j]
    E[i,j] = exp((1 - dist/(m[j]+eps)) / 0.5)
    Z[j]  = sum_i E[i, j]
    r[i]  = max_j E[i,j] / Z[j]
    sim   = mean_i r[i];  loss_dir = mean_b(-log sim)
    out = (loss(s,t) + loss(t,s)) / 2

Sharding: for each of the 4 (batch, direction) calls the distance matrix is
striped by columns across 8 cores (512 columns each).  Each core holds full
columns, so per-column min and Z are exact locally; only the final max over
columns is partial: the device reduces over its own 128-column q-tiles
(GpSimd partition_all_reduce) and the host finishes the max over
(cores x qtiles).

Device layout per (call, qtile): D[q=128 cols (partitions), i=4096 rows
(free)].  PSUM gets the K=128 main matmul (X pre-scaled by -2, bf16) plus
asq[i] via K=1 seed matmuls (operands parked at partitions 0/64; matmul
operand base partitions are restricted to {0,32,64}).  PSUM is drained in
halves: ScalarE Identity(+bsq bias) and VectorE tensor_scalar(+bsq,
min-accum) -> D fp16 in SBUF.  ScalarE Exp(scale=-2/m, bias=2) accumulates
Z; VectorE scales esc by 1/Z (4x mode); GpSimd partition_all_reduce(max)
gives the per-row max over the 128 local columns; one staging row per qtile
(partitions 0/32/64/96) is DMA'd out per call as [4, 4096] fp16.

Calls are software-pipelined (phase 1 of call c+1 is emitted before phase 2
of call c) so the PSUM drains outrank phase-2 work in the Tile scheduler's
program-order priority; this removes all mid-kernel ScalarE gaps.

Cost-model simulated time (CoreSim, per core): ~109.8 us, vs ~133.3 us for
the previous H-matrix-to-host version.  Engine busy: ACT ~92 (Identity
drains 30 + Exp 61), DVE ~83, PE ~56, Pool ~55, DMA ~42.  The packed
ScalarE train is the structural floor: drains and exp are 1x-rate on
ACT/DVE no matter how they are shuffled (PSUM operands disable DVE perf
modes, ScalarE has no perf modes).
"""

import ml_dtypes
import numpy as np

import concourse.bacc as bacc
import concourse.bass as bass
import concourse.tile as tile
from concourse import bass_isa, mybir
from concourse.bass_utils import run_bass_kernel_spmd

N_CORES = 8
C = 128            # channels = matmul contraction dim
NP = 4096          # rows (i) per distance matrix
QS = 512           # column stripe per core
NQT = QS // 128    # 4 q part-tiles per stripe
NCALL = 4          # (batch, direction) pairs
EPS = 1e-5

F32 = mybir.dt.float32
F32R = mybir.dt.float32r
FP16 = mybir.dt.float16
BF16 = mybir.dt.bfloat16
ALU = mybir.AluOpType
ACT = mybir.ActivationFunctionType

LAST_RESULT = None  # BassKernelResults of the most recent run (for test harness)
_NC_CACHE = None


def _build_bass():
    nc = bacc.Bacc(
        "TRN2", target_bir_lowering=False, debug=False, num_devices=N_CORES
    )
    a_d = nc.dram_tensor("a", [NCALL, C, NP], BF16, kind="ExternalInput").ap()
    b2_d = nc.dram_tensor("b2", [NCALL, C, QS], BF16, kind="ExternalInput").ap()
    asq_d = nc.dram_tensor("asq", [NCALL, 2, NP // 2], BF16, kind="ExternalInput").ap()
    bsq_d = nc.dram_tensor("bsq", [NCALL, 128, NQT], F32, kind="ExternalInput").ap()
    ones_d = nc.dram_tensor("ones", [2, 128], BF16, kind="ExternalInput").ap()
    o_d = nc.dram_tensor("o", [NCALL, NQT, NP], FP16, kind="ExternalOutput").ap()

    with tile.TileContext(nc) as tc:
        with (
            tc.tile_pool(name="io", bufs=3) as io_pool,
            tc.tile_pool(name="const", bufs=1) as const_pool,
            tc.tile_pool(name="dtile", bufs=2) as d_pool,
            tc.tile_pool(name="etile", bufs=2) as e_pool,
            tc.tile_pool(name="stats", bufs=10) as st_pool,
            tc.tile_pool(name="stage", bufs=2) as stg_pool,
            tc.tile_pool(name="psum", bufs=2, space="PSUM") as ps_pool,
        ):
            ones = const_pool.tile([128, 128], BF16)
            nc.sync.dma_start(ones[0:128:64, :], ones_d[:])
            bias2 = const_pool.tile([128, 1], F32)
            nc.vector.memset(bias2[:], 2.0)

            def phase1(c):
                # small tensors first; a half 0 before asq so the first main
                # matmuls (issued before the seeds) start as early as possible
                bsq_sb = io_pool.tile([128, NQT], F32, tag="bsq")
                nc.sync.dma_start(bsq_sb[:], bsq_d[c])
                b2_sb = io_pool.tile([C, QS], BF16, tag="b2")
                nc.sync.dma_start(b2_sb[:], b2_d[c])
                a_sb = io_pool.tile([C, NP], BF16, tag="a")
                nc.sync.dma_start(a_sb[:, 0 : NP // 2], a_d[c][:, 0 : NP // 2])
                nc.sync.dma_start(a_sb[:, NP // 2 : NP], a_d[c][:, NP // 2 : NP])
                # asq halves -> partitions 0 and 64 (strided-partition DMA)
                asq_sb = io_pool.tile([128, NP // 2], BF16, tag="asq")
                nc.sync.dma_start(asq_sb[0:128:64, :], asq_d[c])

                # matmuls, PSUM drains, per-column min
                dts, rm2s = [], []
                ACTW = NP // 2  # ScalarE drain share of the h0 psum chunk
                for qt in range(NQT):
                    dt_ = d_pool.tile([128, NP], FP16, tag=f"d{qt}")
                    dts.append(dt_)
                    nmh = 3 if ACTW < NP // 2 else 2
                    mh = st_pool.tile([128, nmh], F32, tag="mh")
                    for h in range(2):
                        ps = ps_pool.tile([128, NP // 2], F32, tag="ps")
                        bp = 0 if h == 0 else 64
                        for ch in range(4):
                            col0 = h * (NP // 2) + ch * 512
                            nc.tensor.matmul(
                                ps[:, ch * 512 : (ch + 1) * 512],
                                lhsT=b2_sb[:, qt * 128 : (qt + 1) * 128],
                                rhs=a_sb[:, col0 : col0 + 512],
                                start=True,
                                stop=False,
                            )
                        for ch in range(4):
                            nc.tensor.matmul(
                                ps[:, ch * 512 : (ch + 1) * 512],
                                lhsT=ones[bp : bp + 1, :],
                                rhs=asq_sb[bp : bp + 1, ch * 512 : (ch + 1) * 512],
                                start=False,
                                stop=True,
                            )
                        if h == 0:
                            # ScalarE drain: D = psum + bsq (fp16 out)
                            nc.scalar.activation(
                                dt_[:, 0:ACTW],
                                ps[:, 0:ACTW],
                                ACT.Identity,
                                bias=bsq_sb[:, qt : qt + 1],
                                scale=1.0,
                            )
                            if ACTW < NP // 2:
                                # VectorE drains the rest of this chunk
                                nc.vector.tensor_scalar(
                                    dt_[:, ACTW : NP // 2],
                                    ps[:, ACTW : NP // 2],
                                    scalar1=bsq_sb[:, qt : qt + 1],
                                    scalar2=None,
                                    op0=ALU.add,
                                    op1=ALU.min,
                                    accum_out=mh[:, 2:3],
                                )

                            # min over the ScalarE share (4x fp16, dummy out)
                            # emitted before the h1 drain so it doesn't queue
                            # behind it on VectorE
                            scr = e_pool.tile([128, ACTW], FP16, tag="scr")
                            nc.vector.tensor_scalar(
                                scr[:],
                                dt_[:, 0:ACTW],
                                scalar1=1.0,
                                scalar2=None,
                                op0=ALU.mult,
                                op1=ALU.min,
                                accum_out=mh[:, 0:1],
                            )
                        else:
                            # VectorE drain: D = psum + bsq with min-accum
                            nc.vector.tensor_scalar(
                                dt_[:, NP // 2 : NP],
                                ps[:],
                                scalar1=bsq_sb[:, qt : qt + 1],
                                scalar2=None,
                                op0=ALU.add,
                                op1=ALU.min,
                                accum_out=mh[:, 1:2],
                            )
                    # accum max of -mh/2 = -(min mh)/2;  rm2 = -2/m
                    # (eps=1e-5 is negligible against m ~ O(100))
                    vd = st_pool.tile([128, nmh], F32, tag="vd")
                    v = st_pool.tile([128, 1], F32, tag="v")
                    nc.vector.tensor_scalar(
                        vd[:], mh[:], scalar1=-0.5, scalar2=None,
                        op0=ALU.mult, op1=ALU.max, accum_out=v[:],
                    )
                    rm2 = st_pool.tile([128, 1], F32, tag="rm2")
                    nc.vector.reciprocal(rm2[:], v[:])
                    rm2s.append(rm2)
                return dts, rm2s

            def phase2(c, dts, rm2s):
                # exp/Z, normalize, per-row max, stage
                stg = stg_pool.tile([128, NP], FP16, tag="stg")
                for qt in range(NQT):
                    esc = e_pool.tile([128, NP], FP16, tag="esc")
                    zc = st_pool.tile([128, 1], F32, tag="zc")
                    nc.scalar.activation(
                        esc[:], dts[qt][:], ACT.Exp,
                        bias=bias2[:], scale=rm2s[qt][:],
                        accum_out=zc[:],
                    )
                    zrec = st_pool.tile([128, 1], F32, tag="zrec")
                    nc.vector.reciprocal(zrec[:], zc[:])
                    sim = e_pool.tile([128, NP], FP16, tag="sim")
                    par = e_pool.tile([128, NP], FP16, tag="par")
                    p0 = 32 * qt
                    # last qtile of the last call: quarter-granularity to
                    # shorten the kernel-tail chain
                    nhc = 4 if (c == NCALL - 1 and qt == NQT - 1) else 1
                    hw = NP // nhc
                    for hc in range(nhc):
                        sl = slice(hc * hw, (hc + 1) * hw)
                        # sim = esc / Z  (4x mode)
                        nc.vector.tensor_scalar(
                            sim[:, sl], esc[:, sl], scalar1=zrec[:],
                            scalar2=None, op0=ALU.mult,
                        )
                        # per-row max over this qtile's 128 columns
                        nc.gpsimd.partition_all_reduce(
                            par[:, sl], sim[:, sl], channels=128,
                            reduce_op=bass_isa.ReduceOp.max,
                        )
                        nc.vector.tensor_copy(
                            stg[p0 : p0 + 1, sl], par[p0 : p0 + 1, sl]
                        )
                if c == NCALL - 1:
                    # overlap most of the final output DMA with the tail
                    nc.sync.dma_start(
                        o_d[c][:, 0 : NP // 2], stg[0:128:32, 0 : NP // 2]
                    )
                    nc.sync.dma_start(
                        o_d[c][:, NP // 2 : NP], stg[0:128:32, NP // 2 : NP]
                    )
                else:
                    nc.sync.dma_start(o_d[c], stg[0:128:32, :])

            # software pipeline across calls: emit phase 1 of call c before
            # phase 2 of call c-1 so the critical drains outrank phase-2 work
            # in the scheduler's program-order priority.
            prev = None
            for c in range(NCALL):
                state = phase1(c)
                if prev is not None:
                    phase2(c - 1, *prev)
                prev = state
            phase2(NCALL - 1, *prev)
    nc.compile()
    return nc


def kernel(source, target):
    global LAST_RESULT
    source = np.ascontiguousarray(np.asarray(source), dtype=np.float32)
    target = np.ascontiguousarray(np.asarray(target), dtype=np.float32)
    B = source.shape[0]
    s = source.reshape(B, C, NP)
    t = target.reshape(B, C, NP)

    # call order: (b0, dir s->t), (b0, dir t->s), (b1, ...), ...
    # dir s->t == _similarity(source, target): cols X = source, rows Y = target
    pairs = []
    for b in range(B):
        pairs.append((s[b], t[b]))
        pairs.append((t[b], s[b]))
    A = np.stack([y for (_x, y) in pairs])                      # [4, 128, 4096]
    X = np.stack([x for (x, _y) in pairs])                      # [4, 128, 4096]
    asq = (A.astype(np.float64) ** 2).sum(axis=1)               # [4, 4096]
    asq2 = np.ascontiguousarray(asq.reshape(NCALL, 2, NP // 2)).astype(
        ml_dtypes.bfloat16
    )
    xsq = (X.astype(np.float64) ** 2).sum(axis=1)               # [4, 4096]
    A16 = A.astype(ml_dtypes.bfloat16)

    in_maps = []
    for k in range(N_CORES):
        sl = slice(k * QS, (k + 1) * QS)
        b2 = np.ascontiguousarray(-2.0 * X[:, :, sl]).astype(ml_dtypes.bfloat16)
        bsq = np.ascontiguousarray(
            xsq[:, sl].reshape(NCALL, NQT, 128).transpose(0, 2, 1)
        ).astype(np.float32)
        in_maps.append({
            "a": A16, "b2": b2, "asq": asq2, "bsq": bsq,
            "ones": np.ones((2, 128), dtype=ml_dtypes.bfloat16),
        })

    global _NC_CACHE
    if _NC_CACHE is None:
        _NC_CACHE = _build_bass()
    nc = _NC_CACHE
    res = run_bass_kernel_spmd(nc, in_maps, core_ids=list(range(N_CORES)))
    LAST_RESULT = res

    # o: [cores][NCALL, NQT, NP] fp16 partial row-maxima
    outs = np.stack([r["o"].astype(np.float64) for r in res.results])
    r = outs.max(axis=(0, 2))                        # [4, 4096]
    sims = r.mean(axis=1)                            # [4]
    loss = float((-np.log(sims)).mean())
    return np.array(loss, dtype=np.float32)
